# revision 3
# baseline (speedup 1.0000x reference)
"""Trainium2 Bass kernel for BYO-GPT v2: sequence-parallel transformer + vocab-parallel unembed.

Sharding (8 cores): core c -> batch b = c // 4, group rank r = c % 4.
Groups [0-3] and [4-7] each run one batch SEQUENCE-PARALLEL: core r owns token
chunks {r, 7-r} (2 x 128 tokens). Per layer each core computes q/k/v/linear/LN
for its own 256 tokens; K and V (bf16) are AllGathered in ONE merged collective
per layer. Causality is a per-core multiplicative 0/1 mask input applied
post-exp (the program is shared across cores).

v2 changes vs v1: bf16 weights (half the HBM traffic), one merged KV AllGather
per layer (collectives serialize on one stream; two ops cost ~56us/layer),
head-pair row-packed score matmuls, 512-wide batched exp, post-exp bf16 masks,
PV with probs as the stationary operand (natural-layout output, no PE
transposes after attention), LayerNorm via fused scalar_tensor_tensor, and a
transposed unembed ([vocab,tokens] tiles) so the per-vocab bias rides the
PSUM->SBUF copy as a per-partition ACT bias; logits emitted in bf16.
"""

import os
import sys

for _p in ("/opt/trn_rl_repo", "/root/.axon_site", "/root/.axon_site/_ro/trn_rl_repo",
           "/root/.axon_site/_ro/pypackages"):
    if os.path.isdir(_p) and _p not in sys.path:
        sys.path.append(_p)

import numpy as np

import concourse.bass as bass
import concourse.mybir as mybir
import concourse.tile as tile
from concourse import bacc
from concourse.bass_utils import run_bass_kernel_spmd
from concourse.masks import make_identity

F32 = mybir.dt.float32
BF16 = mybir.dt.bfloat16
AF = mybir.ActivationFunctionType
ALU = mybir.AluOpType

P = 128
D = 768
KC = D // P          # 6 d-chunks
NHEAD = 12
DH = 64
NPAIR = 6            # head pairs (2 heads / 128 partitions)
EPS = 1e-5
VOCAB = 50257
B = 2
S = 1024
TC = S // P          # 8 token chunks
NOWN = 256           # tokens owned per core (2 chunks)
NCORES = 8
GROUPS = [[0, 1, 2, 3], [4, 5, 6, 7]]
VPAD = 50688         # 396 * 128, divisible by 4
VSH = VPAD // 4      # 12672 per-core vocab shard
VT = VSH // P        # 99 vocab tiles per core
PTW = 4 * NOWN + 4 * P          # 1536 score cols per head
OFFJ = [jc * NOWN if jc < 4 else 4 * NOWN + (jc - 4) * P for jc in range(TC)]
KVW = NPAIR * P * NOWN          # bf16 elems in the K (and V) half of the KV bounce


def _bcast(ap_1d, p=P):
    return bass.AP(tensor=ap_1d.tensor, offset=ap_1d.offset,
                   ap=[[0, p]] + [list(x) for x in ap_1d.ap])


BUILD_VER = 33  # bump on every program change (axon executable cache keys on HLO shape)


def build_program(L=4):
    nc = bacc.Bacc("TRN2", target_bir_lowering=False, debug=False, num_devices=NCORES)

    # ---- DRAM I/O ----
    vtag = nc.dram_tensor("vtag", [1, BUILD_VER], F32, kind="ExternalInput")
    x0pe = nc.dram_tensor("x0pe", [2, P, D], F32, kind="ExternalInput")
    x0T = nc.dram_tensor("x0T", [P, KC, NOWN], BF16, kind="ExternalInput")
    wqT = nc.dram_tensor("wqT", [L, D, D], BF16, kind="ExternalInput")
    wkT = nc.dram_tensor("wkT", [L, D, D], BF16, kind="ExternalInput")
    wvT = nc.dram_tensor("wvT", [L, D, D], BF16, kind="ExternalInput")
    wlT = nc.dram_tensor("wlT", [L, D, D], BF16, kind="ExternalInput")
    bqk = nc.dram_tensor("bqk", [L, 2, D], F32, kind="ExternalInput")   # wq_b, wk_b
    bvl = nc.dram_tensor("bvl", [L, 2, D], F32, kind="ExternalInput")   # wv_b, lin_b
    lnb = nc.dram_tensor("lnb", [L, 4, D], F32, kind="ExternalInput")   # s1,b1,s2,b2
    uT = nc.dram_tensor("uT", [D, VSH], BF16, kind="ExternalInput")
    ub = nc.dram_tensor("ub", [VSH], F32, kind="ExternalInput")
    maskc = nc.dram_tensor("maskc", [TC, P, P], BF16, kind="ExternalInput")  # 1/0
    logitsT = nc.dram_tensor("logitsT", [VSH, S], BF16, kind="ExternalOutput")

    with tile.TileContext(nc) as tc_:
        from contextlib import ExitStack
        with ExitStack() as ctx:
            const = ctx.enter_context(tc_.tile_pool(name="const", bufs=1))
            xpool = ctx.enter_context(tc_.tile_pool(name="xpool", bufs=1))
            lctx = ctx.enter_context(ExitStack())
            ps512 = lctx.enter_context(tc_.tile_pool(name="ps512", bufs=6, space="PSUM"))
            psatt = lctx.enter_context(tc_.tile_pool(name="psatt", bufs=2, space="PSUM"))
            xtp = lctx.enter_context(tc_.tile_pool(name="xtp", bufs=2))
            anpool = lctx.enter_context(tc_.tile_pool(name="anpool", bufs=1))
            wfull = lctx.enter_context(tc_.tile_pool(name="wfull", bufs=3))
            wkp = lctx.enter_context(tc_.tile_pool(name="wkp", bufs=2))
            wqp = lctx.enter_context(tc_.tile_pool(name="wqp", bufs=2))
            qkp = lctx.enter_context(tc_.tile_pool(name="qkp", bufs=1))
            kta = lctx.enter_context(tc_.tile_pool(name="kta", bufs=1))
            vap = lctx.enter_context(tc_.tile_pool(name="vap", bufs=1))
            ptp = lctx.enter_context(tc_.tile_pool(name="ptp", bufs=2))
            lnp = lctx.enter_context(tc_.tile_pool(name="lnp", bufs=1))
            ztp = lctx.enter_context(tc_.tile_pool(name="ztp", bufs=2))
            stp = lctx.enter_context(tc_.tile_pool(name="stp", bufs=6))
            dkv_in = lctx.enter_context(tc_.tile_pool(name="dkvin", bufs=2, space="DRAM"))
            dkv_out = lctx.enter_context(tc_.tile_pool(name="dkvout", bufs=2, space="DRAM"))

            ident = const.tile([P, P], F32)
            make_identity(nc, ident)
            maskt = const.tile([P, TC, P], BF16)
            nc.sync.dma_start(maskt[:], maskc.rearrange("tc p n -> p tc n"))
            eps_t = const.tile([P, 1], F32)
            nc.vector.memset(eps_t[:], EPS)
            vt_t = const.tile([1, BUILD_VER], F32)
            nc.sync.dma_start(vt_t[:], vtag[:])

            # ---- embedding (host pre-added PE): own 2 chunks; host also sends x0T ----
            xcat = xpool.tile([P, 2, D], F32, name="xcat")
            nc.sync.dma_start(xcat[:], x0pe.rearrange("t p d -> p t d"))
            x_To0 = xtp.tile([P, KC, NOWN], BF16, tag="xTo")
            nc.sync.dma_start(x_To0[:], x0T[:])

            def transpose_chunk(dst_xt, t):
                """PE-transpose one x chunk [t,d] -> dst bf16 cols ([d,t]); psum
                slots come from the shared ps512 pool so up to 6 are in flight."""
                for k in range(KC):
                    pt = ps512.tile([P, 512], F32, tag="ps512", name="pt")
                    nc.tensor.transpose(pt[:, 0:P], xcat[:, t, k * P:(k + 1) * P], ident[:])
                    nc.vector.tensor_copy(dst_xt[:, k, t * P:(t + 1) * P], pt[:, 0:P])

            def transpose_own(dst_xt):
                for t in range(2):
                    transpose_chunk(dst_xt, t)

            def layernorm_chunk(t, s_b, b_b):
                xc = xcat[:, t, :]
                stats = stp.tile([P, 3, 6], F32)
                for g in range(3):
                    nc.vector.bn_stats(stats[:, g, :], xcat[:, t, g * 256:(g + 1) * 256])
                mv = stp.tile([P, 2], F32)
                nc.vector.bn_aggr(mv[:], stats[:])
                sd = stp.tile([P, 1], F32)
                nc.scalar.activation(sd[:], mv[:, 1:2], AF.Sqrt, bias=eps_t[:], scale=1.0)
                rs = stp.tile([P, 1], F32)
                nc.vector.reciprocal(rs[:], sd[:])
                tmp = ztp.tile([P, D], F32, tag="lntmp")
                nc.vector.scalar_tensor_tensor(tmp[:], xc, mv[:, 0:1], s_b,
                                               ALU.subtract, ALU.mult)
                nc.vector.scalar_tensor_tensor(xc, tmp[:], rs[:], b_b,
                                               ALU.mult, ALU.add)

            x_To = x_To0
            for l in range(L):
                lnt = lnp.tile([P, 4, D], F32, tag="lnb")
                nc.sync.dma_start(lnt[:], _bcast(lnb[l]))
                bvlt = lnp.tile([P, 2, D], F32, tag="bvl")
                nc.sync.dma_start(bvlt[:], _bcast(bvl[l]))
                bqk_t = lnp.tile([P, 2, NPAIR], F32, tag="bqk")
                nc.sync.dma_start(bqk_t[:], bqk[l].rearrange("t (c p) -> p t c", p=P))

                kv_in = dkv_in.tile([2, NPAIR, P, NOWN], BF16, tag="kvin")
                kv_out = dkv_out.tile([4, 2, NPAIR, P, NOWN], BF16, tag="kvout")

                # ---- K (own tokens, all pairs) -> bounce ----
                wk_s = wkp.tile([P, KC, D], BF16, tag="wk")
                nc.sync.dma_start(wk_s[:], wkT[l].rearrange("(k p) o -> p k o", p=P))
                kts = qkp.tile([P, NPAIR, NOWN], BF16, tag="kts", name="kts")
                for pr in range(NPAIR):
                    pq = ps512.tile([P, 512], F32, tag="ps512")
                    for k in range(KC):
                        nc.tensor.matmul(pq[:, :NOWN], wk_s[:, k, pr * P:(pr + 1) * P],
                                         x_To[:, k, :], start=(k == 0), stop=(k == KC - 1))
                    nc.scalar.activation(kts[:, pr, :], pq[:, :NOWN], AF.Identity,
                                         bias=bqk_t[:, 1, pr:pr + 1], scale=1.0)
                nc.sync.dma_start(kv_in[0].rearrange("s p t -> p s t"), kts[:])

                # ---- V (own tokens, all heads) -> bounce; head-major + ones col ----
                wv_s = wfull.tile([P, KC, D], BF16, tag="wbig")
                nc.sync.dma_start(wv_s[:], wvT[l].rearrange("(k p) o -> p k o", p=P))
                v_own = [qkp.tile([P, NHEAD, DH + 1], BF16, tag=f"vo{t}", name=f"vo{t}")
                         for t in range(2)]
                for t in range(2):
                    nc.vector.memset(v_own[t][:, :, DH:DH + 1], 1.0)
                    for os_, ow in ((0, 512), (512, 256)):
                        pv = ps512.tile([P, 512], F32, tag="ps512")
                        for k in range(KC):
                            nc.tensor.matmul(pv[:, :ow], x_To[:, k, t * P:(t + 1) * P],
                                             wv_s[:, k, os_:os_ + ow],
                                             start=(k == 0), stop=(k == KC - 1))
                        nh = ow // DH
                        nc.vector.tensor_tensor(
                            v_own[t][:, os_ // DH:os_ // DH + nh, 0:DH],
                            pv[:, :ow].rearrange("p (h d) -> p h d", d=DH),
                            bvlt[:, 0, os_:os_ + ow].rearrange("p (h d) -> p h d", d=DH),
                            ALU.add)
                    for s3 in range(3):
                        nc.sync.dma_start(
                            kv_in[1, 3 * t + s3].rearrange("p (hh d) -> p hh d", d=DH),
                            v_own[t][:, 4 * s3:4 * s3 + 4, 0:DH])

                # ---- ONE merged AllGather for K+V within the 4-core group ----
                nc.gpsimd.collective_compute(
                    "AllGather", ALU.bypass, replica_groups=GROUPS,
                    ins=[kv_in[:].opt()], outs=[kv_out[:].opt()])

                # ---- Q (own tokens, all pairs) — overlaps the collective ----
                wq_s = wqp.tile([P, KC, D], BF16, tag="wq")
                nc.sync.dma_start(wq_s[:], wqT[l].rearrange("(k p) o -> p k o", p=P))
                qts = qkp.tile([P, NPAIR, NOWN], BF16, tag="qts", name="qts")
                for pr in range(NPAIR):
                    pq = ps512.tile([P, 512], F32, tag="ps512")
                    for k in range(KC):
                        nc.tensor.matmul(pq[:, :NOWN], wq_s[:, k, pr * P:(pr + 1) * P],
                                         x_To[:, k, :], start=(k == 0), stop=(k == KC - 1))
                    nc.scalar.activation(qts[:, pr, :], pq[:, :NOWN], AF.Identity,
                                         bias=bqk_t[:, 0, pr:pr + 1], scale=1.0)

                # ---- reassemble gathered K/V ----
                kT_all = kta.tile([P, NPAIR, S], BF16, tag="kta")
                v_all = vap.tile([P, TC, NHEAD, DH + 1], BF16, tag="vall")
                nc.vector.memset(v_all[:, :, :, DH:DH + 1], 1.0)
                for jc in range(TC):
                    rr = min(jc, 7 - jc)
                    slot = 0 if jc < 4 else 1
                    nc.sync.dma_start(
                        kT_all[:, :, jc * P:(jc + 1) * P],
                        kv_out[rr, 0, :, :, slot * P:(slot + 1) * P].rearrange("k p t -> p k t"))
                    for s3 in range(3):
                        nc.sync.dma_start(
                            v_all[:, jc, 4 * s3:4 * s3 + 4, 0:DH],
                            kv_out[rr, 1, 3 * slot + s3].rearrange(
                                "p (hh d) -> p hh d", d=DH))

                attn_nat = [anpool.tile([P, D], F32, tag=f"an{t}", name=f"an{t}")
                            for t in range(2)]

                # ---- scores + softmax + PV, head pairs row-packed ----
                for pr in range(NPAIR):
                    p_t = [ptp.tile([P, PTW], BF16, tag=f"pt{hh}", name=f"pt{hh}") for hh in range(2)]
                    for g, jcs in ((0, (0, 1)), (1, (2, 3)), (2, (4, 5, 6, 7))):
                        psg = [ps512.tile([P, 512], F32, tag="ps512", name=f"psg{g}_{i}") for i in range(2)]
                        for jc in jcs:
                            for hh in range(2):
                                hs = DH * hh
                                if jc < 4:
                                    nc.tensor.matmul(
                                        psg[hh][:, (jc - jcs[0]) * NOWN:(jc - jcs[0] + 1) * NOWN],
                                        kT_all[hs:hs + DH, pr, jc * P:(jc + 1) * P],
                                        qts[hs:hs + DH, pr, :],
                                        start=True, stop=True)
                                else:
                                    nc.tensor.matmul(
                                        psg[hh][:, (jc - 4) * P:(jc - 3) * P],
                                        kT_all[hs:hs + DH, pr, jc * P:(jc + 1) * P],
                                        qts[hs:hs + DH, pr, P:NOWN],
                                        start=True, stop=True)
                        for hh in range(2):
                            nc.scalar.activation(p_t[hh][:, g * 512:(g + 1) * 512],
                                                 psg[hh][:], AF.Exp, scale=0.125)
                    for hh in range(2):
                        # zero masked regions: slot0 cols of jc<4, and jc>=4 block
                        v0 = p_t[hh][:, 0:4 * NOWN].rearrange("p (j s q) -> p j s q", s=2, q=P)
                        nc.vector.tensor_mul(v0[:, :, 0, :], v0[:, :, 0, :], maskt[:, 0:4, :])
                        nc.vector.tensor_mul(p_t[hh][:, 4 * NOWN:PTW],
                                             p_t[hh][:, 4 * NOWN:PTW],
                                             maskt[:, 4:8, :].rearrange("p j q -> p (j q)"))
                        # PV: probs stationary -> natural [q, dh+1] with denominator col
                        h = 2 * pr + hh
                        pat = psatt.tile([P, 2, DH + 1], F32, tag="pat")
                        for jc in range(4):
                            nc.tensor.matmul(pat[:, 0, :],
                                             p_t[hh][:, OFFJ[jc]:OFFJ[jc] + P],
                                             v_all[:, jc, h, :],
                                             start=(jc == 0), stop=(jc == 3))
                        for jc in range(TC):
                            o_ = OFFJ[jc] + P if jc < 4 else OFFJ[jc]
                            nc.tensor.matmul(pat[:, 1, :],
                                             p_t[hh][:, o_:o_ + P],
                                             v_all[:, jc, h, :],
                                             start=(jc == 0), stop=(jc == TC - 1))
                        for t in range(2):
                            r_ = stp.tile([P, 1], F32)
                            nc.vector.reciprocal(r_[:], pat[:, t, DH:DH + 1])
                            nc.vector.tensor_scalar_mul(attn_nat[t][:, h * DH:(h + 1) * DH],
                                                        pat[:, t, 0:DH], r_[:])

                # ---- residual + LN1 (own chunks), transposes interleaved ----
                x1_To = xtp.tile([P, KC, NOWN], BF16, tag="x1To")
                for t in range(2):
                    nc.gpsimd.tensor_add(xcat[:, t, :], xcat[:, t, :], attn_nat[t][:])
                    layernorm_chunk(t, lnt[:, 0, :], lnt[:, 1, :])
                    transpose_chunk(x1_To, t)
                wl_s = wfull.tile([P, KC, D], BF16, tag="wbig")
                nc.sync.dma_start(wl_s[:], wlT[l].rearrange("(k p) o -> p k o", p=P))
                x_next = xtp.tile([P, KC, NOWN], BF16, tag="xTo", name="x_next")
                for t in range(2):
                    zt = ztp.tile([P, D], F32, tag="zt")
                    for os_, ow in ((0, 512), (512, 256)):
                        pl_ = ps512.tile([P, 512], F32, tag="ps512")
                        for k in range(KC):
                            nc.tensor.matmul(pl_[:, :ow], x1_To[:, k, t * P:(t + 1) * P],
                                             wl_s[:, k, os_:os_ + ow],
                                             start=(k == 0), stop=(k == KC - 1))
                        nc.vector.tensor_tensor(zt[:, os_:os_ + ow], pl_[:, :ow],
                                                bvlt[:, 1, os_:os_ + ow], ALU.add)
                    nc.gpsimd.tensor_add(xcat[:, t, :], xcat[:, t, :], zt[:])
                    layernorm_chunk(t, lnt[:, 2, :], lnt[:, 3, :])
                    transpose_chunk(x_next, t)

                # transposed x for next layer's QKV (or the unembed after layer L-1)
                x_To = x_next

            # ---- gather x4 (bf16, transposed) then vocab-sharded unembed ----
            x4in = dkv_in.tile([KC, P, NOWN], BF16, tag="x4in")
            x4out = dkv_out.tile([4, KC, P, NOWN], BF16, tag="x4out")
            nc.sync.dma_start(x4in[:].rearrange("k p t -> p k t"), x_To[:])
            nc.gpsimd.collective_compute(
                "AllGather", ALU.bypass, replica_groups=GROUPS,
                ins=[x4in[:].opt()], outs=[x4out[:].opt()])
            x4_T = kta.tile([P, KC, S], BF16, tag="x4T")
            for jc in range(TC):
                rr = min(jc, 7 - jc)
                slot = 0 if jc < 4 else 1
                nc.sync.dma_start(
                    x4_T[:, :, jc * P:(jc + 1) * P],
                    x4out[rr, :, :, slot * P:(slot + 1) * P].rearrange("k p t -> p k t"))

            lctx.close()
            psu = ctx.enter_context(tc_.tile_pool(name="psu", bufs=6, space="PSUM"))
            upool = ctx.enter_context(tc_.tile_pool(name="upool", bufs=4))
            ubp = ctx.enter_context(tc_.tile_pool(name="ubp", bufs=1))
            lop = ctx.enter_context(tc_.tile_pool(name="lop", bufs=3))
            uT_t = uT.rearrange("(k p) v -> p k v", p=P)
            ubt = ubp.tile([P, VT], F32)
            nc.sync.dma_start(ubt[:], ub.rearrange("(v p) -> p v", p=P))
            # [vocab-tile 128, tokens] output tiles: per-vocab bias is per-partition,
            # fused into the PSUM->SBUF copy (ACT / DVE alternating).
            for vt in range(VT):
                u_s = upool.tile([P, KC, P], BF16, tag="u")
                nc.sync.dma_start(u_s[:], uT_t[:, :, vt * P:(vt + 1) * P])
                lo = lop.tile([P, 2, 512], BF16, tag="lo")
                for th in range(2):
                    pu = psu.tile([P, 512], F32, tag="psu")
                    for k in range(KC):
                        nc.tensor.matmul(pu[:], u_s[:, k, :],
                                         x4_T[:, k, th * 512:(th + 1) * 512],
                                         start=(k == 0), stop=(k == KC - 1))
                    if th == 0:
                        nc.vector.tensor_scalar_add(lo[:, th, :], pu[:], ubt[:, vt:vt + 1])
                    else:
                        nc.scalar.activation(lo[:, th, :], pu[:], AF.Identity,
                                             bias=ubt[:, vt:vt + 1], scale=1.0)
                nc.sync.dma_start(logitsT[vt * P:(vt + 1) * P, :],
                                  lo[:].rearrange("p t x -> p (t x)"))

    nc.compile()
    return nc


_CACHE = {}


def get_program(L=4, dbg=False):
    key = (L, dbg)
    if key not in _CACHE:
        _CACHE[key] = build_program(L)
    return _CACHE[key]


def make_sp_mask(r):
    """Multiplicative causal mask per j-chunk for core rank r (1=keep, 0=drop):
    jc<4 masks slot0 (q chunk r); jc>=4 masks slot1 (q chunk 7-r)."""
    m = np.empty((TC, P, P), np.float32)
    jl = np.arange(P)
    il = np.arange(P)
    for jc in range(TC):
        chunk = r if jc < 4 else 7 - r
        jg = jc * P + jl[:, None]
        ig = chunk * P + il[None, :]
        m[jc] = np.where(jg <= ig, 1.0, 0.0)
    return m


def make_core_inputs(tokens, embed, pe, wq_w, wq_b, wk_w, wk_b, wv_w, wv_b,
                     lin_w, lin_b, n1_s, n1_b, n2_s, n2_b, unembed_w, unembed_b,
                     L=4):
    c = np.ascontiguousarray
    f = np.float32
    import ml_dtypes
    bf = ml_dtypes.bfloat16
    tokens = np.asarray(tokens)
    embed = np.asarray(embed, f)
    pe_s = np.asarray(pe, f)[:S]
    wqT = c(np.asarray(wq_w, f)[:L].transpose(0, 2, 1).astype(bf))
    wkT = c(np.asarray(wk_w, f)[:L].transpose(0, 2, 1).astype(bf))
    wvT = c(np.asarray(wv_w, f)[:L].transpose(0, 2, 1).astype(bf))
    wlT = c(np.asarray(lin_w, f)[:L].transpose(0, 2, 1).astype(bf))
    upad = np.zeros((4 * VSH, D), f)
    ubpad = np.zeros((4 * VSH,), f)
    nv = min(VOCAB, 4 * VSH, np.asarray(unembed_w).shape[0])
    upad[:nv] = np.asarray(unembed_w, f)[:nv]
    ubpad[:nv] = np.asarray(unembed_b, f)[:nv]
    bqk_h = c(np.stack([np.asarray(wq_b, f)[:L], np.asarray(wk_b, f)[:L]], axis=1))
    bvl_h = c(np.stack([np.asarray(wv_b, f)[:L], np.asarray(lin_b, f)[:L]], axis=1))
    lnb_h = c(np.stack([np.asarray(n1_s, f)[:L], np.asarray(n1_b, f)[:L],
                        np.asarray(n2_s, f)[:L], np.asarray(n2_b, f)[:L]], axis=1))
    common = dict(vtag=np.zeros((1, BUILD_VER), f), wqT=wqT, wkT=wkT, wvT=wvT, wlT=wlT,
                  bqk=bqk_h, bvl=bvl_h, lnb=lnb_h)
    in_maps = []
    for core in range(NCORES):
        b = core // 4
        r = core % 4
        xfull = embed[tokens[b, :S]] + pe_s
        x0pe_h = np.stack([xfull[r * P:(r + 1) * P], xfull[(7 - r) * P:(8 - r) * P]])
        # x0T: [P, KC, NOWN] = transposed own chunks in bf16
        x_own = np.concatenate([x0pe_h[0], x0pe_h[1]], axis=0)      # [256, 768]
        x0T_h = c(x_own.T.reshape(KC, P, NOWN).transpose(1, 0, 2).astype(bf))
        uT_c = c(upad[r * VSH:(r + 1) * VSH].T.astype(bf))
        in_maps.append(dict(common, x0pe=c(x0pe_h.astype(f)), x0T=x0T_h, uT=uT_c,
                            ub=c(ubpad[r * VSH:(r + 1) * VSH]),
                            maskc=c(make_sp_mask(r).astype(bf))))
    return in_maps


def kernel(**inputs):
    nc = get_program(4)
    in_maps = make_core_inputs(**inputs)
    res = run_bass_kernel_spmd(nc, in_maps, core_ids=list(range(NCORES)))
    out = np.zeros((B, S, VOCAB), np.float32)
    for core in range(NCORES):
        b = core // 4
        s_ = core % 4
        lo = res.results[core]["logitsT"]
        v0 = s_ * VSH
        v1 = min(v0 + VSH, VOCAB)
        if v1 > v0:
            out[b, :, v0:v1] = lo[:v1 - v0, :].T.astype(np.float32)
    return out
